# revision 24
# baseline (speedup 1.0000x reference)
"""GAT GNN kernel for 8 Trainium2 NeuronCores (Bass, via PJRT/axon).

Single fused launch for all three GAT layers. Strategy:
  - Nodes are permuted by in-degree so each 128-node dst tile has
    near-uniform degree -> tight ELL slot grids, shared by all layers.
  - Host computes h1 = x @ W1 and es/ed logit terms, ships per-core bf16
    table shards (row = [h(64) | es_hi | es_lo | ed_hi | ed_lo]) plus the
    ELL index grid (uint32 positions into the AllGathered table).
  - Device, per layer: AllGather table shards DRAM->DRAM across the 8
    cores (NeuronLink, not the host tunnel), per-tile indirect-DMA
    gathers, on-chip segment softmax (DVE+ACT), slot reduction, then one
    PE matmul per tile with rhs [W_next | W_next@a_src | W_next@a_dst]
    which directly yields the next layer's [h | es | ed] table shard.
  - Only the final [N, 64] bf16 node features return to host; mean/max
    pooling by graph and the final linear run on host.
Per-layer tables stay on device; host<->device traffic is ~28 MB up +
13 MB down total (the axon tunnel moves ~40 MB/s, so this dominates
wall time; device exec is ~10 ms, Pool-engine SWDGE bound).
"""
import sys

sys.path.insert(0, "/opt/trn_rl_repo")

import numpy as np

P = 128
F = 64                 # hidden width
RC = 68                # table row: 64 feats + es_hi, es_lo, ed_hi, ed_lo
RCO = 66               # matmul output row: 64 feats + es + ed (fp32)
NEG_SLOPE = 0.2
N_CORES = 8
COLS_BUDGET = 224      # gather cols per group
MAX_NT = 8
G = 512                # graphs

_RUNNERS = {}


def _bf16():
    import ml_dtypes
    return ml_dtypes.bfloat16


def _make_runner(nc, replicated_names):
    """jit the bass module over 8 cores via shard_map. Inputs in
    replicated_names get PartitionSpec(None); others shard on axis 0."""
    import jax
    from jax.sharding import Mesh, PartitionSpec
    from jax.experimental.shard_map import shard_map
    import concourse.mybir as mybir
    from concourse.bass2jax import (_bass_exec_p, partition_id_tensor,
                                    install_neuronx_cc_hook)

    install_neuronx_cc_hook()
    nc.finalize()
    partition_name = nc.partition_id_tensor.name if nc.partition_id_tensor else None

    in_names, out_names, out_avals, zero_outs = [], [], [], []
    for alloc in nc.m.functions[0].allocations:
        if not isinstance(alloc, mybir.MemoryLocationSet):
            continue
        name = alloc.memorylocations[0].name
        if alloc.kind == "ExternalInput":
            if name != partition_name:
                in_names.append(name)
        elif alloc.kind == "ExternalOutput":
            shape = tuple(alloc.tensor_shape)
            dtype = mybir.dt.np(alloc.dtype)
            out_names.append(name)
            out_avals.append(jax.core.ShapedArray(shape, dtype))
            zero_outs.append(np.zeros(shape, dtype))
    all_in = in_names + out_names + ([partition_name] if partition_name else [])

    def _body(*args):
        operands = list(args)
        if partition_name is not None:
            operands.append(partition_id_tensor())
        return tuple(_bass_exec_p.bind(
            *operands,
            out_avals=tuple(out_avals), in_names=tuple(all_in),
            out_names=tuple(out_names), lowering_input_output_aliases=(),
            sim_require_finite=False, sim_require_nnan=False, nc=nc))

    devices = jax.devices()[:N_CORES]
    mesh = Mesh(np.asarray(devices), ("core",))
    in_specs = tuple(
        PartitionSpec(None) if n in replicated_names else PartitionSpec("core")
        for n in in_names) + (PartitionSpec("core"),) * len(out_names)
    out_specs = (PartitionSpec("core"),) * len(out_names)
    jfn = jax.jit(shard_map(_body, mesh=mesh, in_specs=in_specs,
                            out_specs=out_specs, check_rep=False),
                  keep_unused=True)

    dev_zeros = []

    def fn(global_ins):
        import jax as _jax
        from jax.sharding import NamedSharding
        args = [global_ins[n] for n in in_names]
        if not dev_zeros:
            # outputs are fully written by the kernel; keep one
            # device-resident zero buffer instead of re-uploading 12.8MB
            # of zeros through the tunnel every call
            sh = NamedSharding(mesh, PartitionSpec("core"))
            for z in zero_outs:
                dev_zeros.append(_jax.device_put(
                    np.zeros((N_CORES * z.shape[0], *z.shape[1:]), z.dtype),
                    sh))
        args += dev_zeros
        outs = jfn(*args)
        _jax.block_until_ready(outs)
        return np.asarray(outs[0])

    return fn, in_names, jfn, replicated_names


def _build_fused_kernel(groups, TOTC, SHR, T_core, GCOLS):
    """All three GAT layers + graph pooling in one module, one core's
    dst shard each. Output: [P, 2F] fp32 (rows 0..G/8-1 = this core's
    graphs; cols 0..F-1 mean-pool, F..2F-1 max-pool)."""
    import concourse.bacc as bacc
    import concourse.bass as bass
    import concourse.mybir as mybir
    import concourse.tile as tile
    from concourse.masks import make_identity

    DT = mybir.dt.float32
    BF = mybir.dt.bfloat16
    A = mybir.AluOpType
    RTOT = N_CORES * SHR

    nc = bacc.Bacc("TRN2", target_bir_lowering=False, debug=False,
                   num_devices=N_CORES)
    t1_d = nc.dram_tensor("t1", [SHR, RC], BF, kind="ExternalInput")
    idx_d = nc.dram_tensor("idx", [P, TOTC], mybir.dt.uint32,
                           kind="ExternalInput")
    esb_d = nc.dram_tensor("esb", [P, T_core], DT, kind="ExternalInput")
    gidx_d = nc.dram_tensor("gidx", [P, GCOLS], mybir.dt.uint32,
                            kind="ExternalInput")
    gfix_d = nc.dram_tensor("gfix", [P, 2], DT, kind="ExternalInput")
    w2_d = nc.dram_tensor("w2", [F, RCO], DT, kind="ExternalInput")
    w3_d = nc.dram_tensor("w3", [F, RCO], DT, kind="ExternalInput")
    b1_d = nc.dram_tensor("b1", [P, F], DT, kind="ExternalInput")
    b2_d = nc.dram_tensor("b2", [P, F], DT, kind="ExternalInput")
    b3_d = nc.dram_tensor("b3", [P, F], DT, kind="ExternalInput")
    out_d = nc.dram_tensor("out", [P, 2 * F], DT, kind="ExternalOutput")

    with tile.TileContext(nc) as tc:
        with (tc.tile_pool(name="const", bufs=1) as cpool,
              tc.tile_pool(name="sb", bufs=2) as pool,
              tc.tile_pool(name="ps", bufs=2, space="PSUM") as pspool,
              tc.tile_pool(name="dS", bufs=2, space="DRAM") as dramS,
              tc.tile_pool(name="dS3", bufs=1, space="DRAM") as dramS3,
              tc.tile_pool(name="dT3", bufs=1, space="DRAM") as dramT3,
              tc.tile_pool(name="dT", bufs=2, space="DRAM") as dramT):
            ident = cpool.tile([P, P], DT)
            make_identity(nc, ident[:])
            w_sb = {}
            for lname, wd in ((2, w2_d), (3, w3_d)):
                w_sb[lname] = cpool.tile([F, RCO], DT, name=f"wsb{lname}")
                nc.sync.dma_start(out=w_sb[lname][:], in_=wd[:])
            b_sb = {}
            for lname, bd in ((1, b1_d), (2, b2_d), (3, b3_d)):
                b_sb[lname] = cpool.tile([P, F], DT, name=f"bsb{lname}")
                nc.sync.dma_start(out=b_sb[lname][:], in_=bd[:])
            esb_sb = cpool.tile([P, T_core], DT)
            nc.sync.dma_start(out=esb_sb[:], in_=esb_d[:])

            # layer-1 table: bounce the input shard, AllGather to full T
            S0 = dramS.tile([SHR, RC], BF, tag="S")
            nc.gpsimd.dma_start(out=S0[:], in_=t1_d[:])
            T_cur = dramT.tile([RTOT, RC], BF, tag="T")
            nc.gpsimd.collective_compute(
                "AllGather", A.bypass,
                replica_groups=[list(range(N_CORES))],
                ins=[S0.opt()], outs=[T_cur.opt()])

            def bcast(t, dims):
                b = t[:]
                return bass.AP(b.tensor, b.offset, [[b.ap[0][0], P]] + dims)

            S3 = None
            for layer in (1, 2, 3):
                S_next = (dramS.tile([SHR, RC], BF, tag="S", name="Snext")
                          if layer < 3 else None)
                if layer == 3:
                    S3 = dramS3.tile([SHR, F], BF, tag="S3", name="S3t")
                assert T_cur[:].offset == 0
                for (col_off, j0, nt, Kg) in groups:
                    cols = nt * Kg
                    it = pool.tile([P, cols], mybir.dt.uint32, tag="it")
                    nc.sync.dma_start(out=it[:],
                                      in_=idx_d[:, col_off:col_off + cols])
                    g = pool.tile([P, cols * RC], BF, tag="g")
                    for cc in range(cols):
                        nc.gpsimd.indirect_dma_start(
                            out=g[:, cc * RC:(cc + 1) * RC], out_offset=None,
                            in_=T_cur[:],
                            in_offset=bass.IndirectOffsetOnAxis(
                                ap=it[:, cc:cc + 1], axis=0))
                    gb = g[:]
                    pstep = gb.ap[0][0]

                    def gap(off, dims):
                        return bass.AP(gb.tensor, gb.offset + off,
                                       [[pstep, P]] + dims)

                    # z = es_hi + es_lo (slot) + ed_hi + ed_lo (self slot 0)
                    z = pool.tile([P, cols], DT, tag="z")
                    nc.vector.tensor_tensor(
                        out=z[:], in0=gap(F, [[RC, cols]]),
                        in1=gap(F + 1, [[RC, cols]]), op=A.add)
                    zed = pool.tile([P, nt], DT, tag="zed")
                    nc.vector.tensor_tensor(
                        out=zed[:], in0=gap(F + 2, [[Kg * RC, nt]]),
                        in1=gap(F + 3, [[Kg * RC, nt]]), op=A.add)
                    nc.vector.tensor_tensor(
                        out=z[:], in0=z[:],
                        in1=bcast(zed, [[1, nt], [0, Kg]]), op=A.add)
                    # leaky relu (exact): z = max(z, 0.2z), clamp at -30
                    zt = pool.tile([P, cols], DT, tag="zt")
                    nc.vector.tensor_scalar_mul(zt[:], z[:], NEG_SLOPE)
                    nc.vector.tensor_tensor(out=z[:], in0=z[:], in1=zt[:],
                                            op=A.max)
                    nc.vector.tensor_scalar_max(z[:], z[:], -30.0)
                    pex = pool.tile([P, cols], BF, tag="pex")
                    nc.scalar.activation(pex[:], z[:],
                                         mybir.ActivationFunctionType.Exp)
                    den = pool.tile([P, nt], DT, tag="den")
                    nc.vector.tensor_reduce(
                        out=den[:],
                        in_=pex[:].rearrange("p (t k) -> p t k", k=Kg),
                        axis=mybir.AxisListType.X, op=A.add)
                    nc.vector.reciprocal(den[:], den[:])
                    # feats *= p (in place, bf16)
                    nc.vector.tensor_tensor(
                        out=gap(0, [[RC, cols], [1, F]]),
                        in0=gap(0, [[RC, cols], [1, F]]),
                        in1=bcast(pex, [[1, cols], [0, F]]),
                        op=A.mult)
                    # reduce slots -> agg [P, nt*F] fp32
                    agg = pool.tile([P, nt * F], DT, tag="agg")
                    nc.vector.tensor_reduce(
                        out=agg[:],
                        in_=gap(0, [[Kg * RC, nt], [1, F], [RC, Kg]]),
                        axis=mybir.AxisListType.X, op=A.add)
                    nc.vector.tensor_tensor(
                        out=agg[:], in0=agg[:],
                        in1=bcast(den, [[1, nt], [0, F]]), op=A.mult)
                    nc.vector.tensor_tensor(
                        out=agg[:], in0=agg[:],
                        in1=bcast(b_sb[layer], [[0, nt], [1, F]]), op=A.add)

                    if layer == 3:
                        outf = pool.tile([P, nt * F], BF, tag="outf")
                        nc.vector.tensor_copy(out=outf[:], in_=agg[:])
                        sb3 = S3[:]
                        dst = bass.AP(sb3.tensor, sb3.offset + j0 * P * F,
                                      [[F, P], [P * F, nt], [1, F]])
                        nc.sync.dma_start(out=dst, in_=outf[:])
                        continue

                    # relu, transpose, matmul -> [h | es | ed] fp32
                    nc.vector.tensor_scalar_max(agg[:], agg[:], 0.0)
                    aggv = agg[:].rearrange("p (t f) -> p t f", f=F)
                    psT = pspool.tile([F, nt * P], DT, tag="psT")
                    for t in range(nt):
                        nc.tensor.transpose(out=psT[:, t * P:(t + 1) * P],
                                            in_=aggv[:, t, :],
                                            identity=ident[:])
                    aggT = pool.tile([F, nt * P], DT, tag="aggT")
                    nc.vector.tensor_copy(out=aggT[:], in_=psT[:])
                    # 128-float (512B) stride per tile so no matmul output
                    # crosses a 2KB PSUM bank boundary
                    RCOP = 128
                    psO = pspool.tile([P, nt * RCOP], DT, tag="psO")
                    for t in range(nt):
                        nc.tensor.matmul(
                            out=psO[:, t * RCOP:t * RCOP + RCO],
                            lhsT=aggT[:, t * P:(t + 1) * P],
                            rhs=w_sb[layer + 1][:], start=True, stop=True)
                    pb = psO[:]
                    pstep2 = pb.ap[0][0]

                    def pap(off, dims):
                        return bass.AP(pb.tensor, pb.offset + off,
                                       [[pstep2, P]] + dims)

                    nxt = pool.tile([P, nt * RC], BF, tag="nxt")
                    nb = nxt[:]
                    nstep = nb.ap[0][0]

                    def nap(off, dims):
                        return bass.AP(nb.tensor, nb.offset + off,
                                       [[nstep, P]] + dims)

                    # feats
                    nc.vector.tensor_copy(
                        out=nap(0, [[RC, nt], [1, F]]),
                        in_=pap(0, [[RCOP, nt], [1, F]]))
                    # es_eff = es + bias (bias -200 on pad rows); hi/lo bf16
                    esum = pool.tile([P, nt], DT, tag="esum")
                    eb = esb_sb[:]
                    nc.vector.tensor_tensor(
                        out=esum[:], in0=pap(F, [[RCOP, nt]]),
                        in1=bass.AP(eb.tensor, eb.offset + j0,
                                    [[eb.ap[0][0], P], [1, nt]]),
                        op=A.add)
                    nc.vector.tensor_copy(out=nap(F, [[RC, nt]]),
                                          in_=esum[:])
                    hif = pool.tile([P, nt], DT, tag="hif")
                    nc.vector.tensor_copy(out=hif[:], in_=nap(F, [[RC, nt]]))
                    nc.vector.tensor_tensor(
                        out=nap(F + 1, [[RC, nt]]), in0=esum[:], in1=hif[:],
                        op=A.subtract)
                    # ed hi/lo
                    nc.vector.tensor_copy(out=nap(F + 2, [[RC, nt]]),
                                          in_=pap(F + 1, [[RCOP, nt]]))
                    hif2 = pool.tile([P, nt], DT, tag="hif2")
                    nc.vector.tensor_copy(out=hif2[:],
                                          in_=nap(F + 2, [[RC, nt]]))
                    nc.vector.tensor_tensor(
                        out=nap(F + 3, [[RC, nt]]),
                        in0=pap(F + 1, [[RCOP, nt]]), in1=hif2[:],
                        op=A.subtract)
                    sb = S_next[:]
                    dst = bass.AP(sb.tensor, sb.offset + j0 * P * RC,
                                  [[RC, P], [P * RC, nt], [1, RC]])
                    nc.sync.dma_start(out=dst, in_=nxt[:])

                if layer < 3:
                    T_cur = dramT.tile([RTOT, RC], BF, tag="T")
                    nc.gpsimd.collective_compute(
                        "AllGather", A.bypass,
                        replica_groups=[list(range(N_CORES))],
                        ins=[S_next.opt()], outs=[T_cur.opt()])

            # ---- graph pooling: gather this core's graphs' node rows ----
            T3 = dramT3.tile([RTOT, F], BF, tag="T3")
            nc.gpsimd.collective_compute(
                "AllGather", A.bypass,
                replica_groups=[list(range(N_CORES))],
                ins=[S3.opt()], outs=[T3.opt()])
            assert T3[:].offset == 0
            git = pool.tile([P, GCOLS], mybir.dt.uint32, tag="git")
            nc.sync.dma_start(out=git[:], in_=gidx_d[:])
            gfix = pool.tile([P, 2], DT, tag="gfix")
            nc.sync.dma_start(out=gfix[:], in_=gfix_d[:])
            gg = pool.tile([P, GCOLS * F], BF, tag="gg")
            for cc in range(GCOLS):
                nc.gpsimd.indirect_dma_start(
                    out=gg[:, cc * F:(cc + 1) * F], out_offset=None,
                    in_=T3[:],
                    in_offset=bass.IndirectOffsetOnAxis(
                        ap=git[:, cc:cc + 1], axis=0))
            ggb = gg[:]
            gstep = ggb.ap[0][0]
            gview = bass.AP(ggb.tensor, ggb.offset,
                            [[gstep, P], [1, F], [F, GCOLS]])
            pooled = pool.tile([P, 2 * F], DT, tag="pooled")
            nc.vector.tensor_reduce(out=pooled[:, 0:F], in_=gview,
                                    axis=mybir.AxisListType.X, op=A.add)
            nc.vector.tensor_reduce(out=pooled[:, F:2 * F], in_=gview,
                                    axis=mybir.AxisListType.X, op=A.max)
            # pad slots duplicated the graph's first node (slot 0):
            # sum -= padcnt * h_first; mean = sum * recip_count
            s0f = pool.tile([P, F], DT, tag="s0f")
            nc.vector.tensor_copy(out=s0f[:], in_=gg[:, 0:F])
            gfb = gfix[:]
            nc.vector.tensor_tensor(
                out=s0f[:], in0=s0f[:],
                in1=bass.AP(gfb.tensor, gfb.offset,
                            [[gfb.ap[0][0], P], [0, F]]), op=A.mult)
            nc.vector.tensor_tensor(out=pooled[:, 0:F], in0=pooled[:, 0:F],
                                    in1=s0f[:], op=A.subtract)
            nc.vector.tensor_tensor(
                out=pooled[:, 0:F], in0=pooled[:, 0:F],
                in1=bass.AP(gfb.tensor, gfb.offset + 1,
                            [[gfb.ap[0][0], P], [0, F]]), op=A.mult)
            nc.sync.dma_start(out=out_d[:], in_=pooled[:])
    return nc


def _prep_light(N, dst):
    """Degree permutation, table positions, ELL group geometry, es bias."""
    deg = np.bincount(dst, minlength=N).astype(np.int32) + 1  # + self loop
    order = np.argsort(deg, kind="stable")
    rank = np.empty(N, np.int32)
    rank[order] = np.arange(N, dtype=np.int32)
    n_tiles = (N + P - 1) // P
    T_core = (n_tiles + N_CORES - 1) // N_CORES
    SHR = T_core * P
    RTOT = N_CORES * SHR

    # rank r -> gathered-table position
    r_all = np.arange(RTOT, dtype=np.int32)
    t_all = r_all >> 7
    posr = ((t_all % N_CORES) * SHR + (t_all // N_CORES) * P
            + (r_all & 127)).astype(np.int32)
    DUMMY = np.uint32(posr[RTOT - 1])

    degp = np.zeros(RTOT, np.int32)
    degp[:N] = deg[order]
    tile_max = degp.reshape(-1, P).max(1)
    K_hat = np.maximum(2, tile_max.reshape(T_core, N_CORES).max(1))

    groups = []
    j = 0
    col = 0
    while j < T_core:
        nt = 1
        kg = int(K_hat[j])
        while (j + nt < T_core and nt < MAX_NT
               and (nt + 1) * max(kg, int(K_hat[j + nt])) <= COLS_BUDGET):
            kg = max(kg, int(K_hat[j + nt]))
            nt += 1
        groups.append((col, j, nt, kg))
        col += nt * kg
        j += nt
    TOTC = col

    col_base = np.empty(T_core, np.int64)
    for (col_off, j0, nt, kg) in groups:
        for t in range(nt):
            col_base[j0 + t] = col_off + t * kg

    # es bias: -200 on pad rows (rank >= N)
    esb = np.zeros((N_CORES, P, T_core), np.float32)
    rp = np.arange(N, RTOT, dtype=np.int64)
    tp = rp >> 7
    esb[tp % N_CORES, rp & 127, tp // N_CORES] = -200.0

    return dict(deg=deg, order=order, rank=rank, posr=posr, T_core=T_core,
                SHR=SHR, RTOT=RTOT, groups=groups, TOTC=TOTC,
                col_base=col_base, DUMMY=DUMMY, esb=esb)


def _prep_edges(gp, N, src, dst):
    """ELL slot grid fill [cores, P, TOTC] (order within a row is free,
    so an unstable int32 argsort suffices)."""
    rank, posr, col_base = gp["rank"], gp["posr"], gp["col_base"]
    TOTC, DUMMY = gp["TOTC"], gp["DUMMY"]
    posn = posr[rank]                       # node id -> table position
    dstr = rank[dst]                        # int32
    srcp = posn[src].astype(np.uint32)
    ord_e = np.argsort(dstr)                # unstable, fast
    dstr_s = dstr[ord_e]
    srcp_s = srcp[ord_e]
    cnt = np.bincount(dstr_s, minlength=N)
    indptr = np.zeros(N + 1, np.int64)
    np.cumsum(cnt, out=indptr[1:])
    k_e = np.arange(len(dstr_s), dtype=np.int64) - indptr[dstr_s]

    ell = np.full((N_CORES, P, TOTC), DUMMY, np.uint32)
    r = np.arange(N, dtype=np.int64)
    t = r >> 7
    ell[t % N_CORES, r & 127, col_base[t // N_CORES]] = \
        posr[:N].astype(np.uint32)  # self slot of rank r
    te = dstr_s >> 7
    ell[te % N_CORES, dstr_s & 127,
        col_base[te // N_CORES] + 1 + k_e] = srcp_s
    return ell


def _prep_graph(N, src, dst):
    """Compatibility wrapper: light + edge prep merged."""
    gp = _prep_light(N, dst)
    gp["ell"] = _prep_edges(gp, N, src, dst)
    return gp


def _hi_lo(v, bf):
    hi = v.astype(bf)
    lo = (v - hi.astype(np.float32)).astype(bf)
    return hi, lo


def kernel(x, edge_index, batch, W1, as1, ad1, b1, W2, as2, ad2, b2,
           W3, as3, ad3, b3, linW, linb):
    import time
    bf = _bf16()

    x = np.asarray(x, np.float32)
    edge_index = np.asarray(edge_index)
    batch = np.asarray(batch).astype(np.int64)
    W1, W2, W3 = (np.asarray(w, np.float32) for w in (W1, W2, W3))
    as1, ad1, as2, ad2, as3, ad3 = (np.asarray(a, np.float32)
                                    for a in (as1, ad1, as2, ad2, as3, ad3))
    b1, b2, b3 = (np.asarray(b, np.float32) for b in (b1, b2, b3))
    linW = np.asarray(linW, np.float32)
    linb = np.asarray(linb, np.float32)

    N = x.shape[0]
    src = edge_index[0].astype(np.int64)
    dst = edge_index[1].astype(np.int64)

    t_prep0 = time.perf_counter()
    gp = _prep_light(N, dst)
    SHR, T_core, RTOT = gp["SHR"], gp["T_core"], gp["RTOT"]
    posr, rank = gp["posr"], gp["rank"]
    posn = posr[rank]       # node id -> table position

    # graph pooling layout: core c owns graphs [c*G/8, (c+1)*G/8)
    GPC = G // N_CORES
    starts = np.searchsorted(batch, np.arange(G))
    ends = np.searchsorted(batch, np.arange(G), side="right")
    counts = (ends - starts).astype(np.int64)
    GCOLS = max(1, int(counts.max()))
    firstpos = np.where(counts > 0,
                        posn[np.minimum(starts, N - 1)],
                        gp["DUMMY"]).astype(np.uint32)
    garr = np.tile(firstpos[:, None], (1, GCOLS))
    n_all = np.arange(N, dtype=np.int64)
    gof = batch
    garr[gof, n_all - starts[gof]] = posn.astype(np.uint32)
    gidx = np.full((N_CORES, P, GCOLS), gp["DUMMY"], np.uint32)
    gidx[:, :GPC, :] = garr.reshape(N_CORES, GPC, GCOLS)
    gfix = np.zeros((N_CORES, P, 2), np.float32)
    gfix[:, :, 0] = GCOLS  # unused rows: all-pad
    gfix[:, :GPC, 0] = (GCOLS - counts).reshape(N_CORES, GPC)
    gfix[:, :, 1] = 1.0
    gfix[:, :GPC, 1] = (1.0 / np.maximum(counts, 1)).reshape(N_CORES, GPC)

    key = (N, int(edge_index.shape[1]), gp["TOTC"], GCOLS)
    cold = key not in _RUNNERS
    if cold:
        nc = _build_fused_kernel(gp["groups"], gp["TOTC"], SHR, T_core,
                                 GCOLS)
        fn, in_names, jfn, repl = _make_runner(
            nc, {"w2", "w3", "b1", "b2", "b3"})
        _RUNNERS[key] = (fn, in_names, jfn, repl)
    fn, in_names, _, _ = _RUNNERS[key]

    import jax
    from jax.sharding import Mesh, PartitionSpec, NamedSharding
    mesh = Mesh(np.asarray(jax.devices()[:N_CORES]), ("core",))
    shard = NamedSharding(mesh, PartitionSpec("core"))

    # layer-1 table shard: [h1 | es_hi | es_lo | ed_hi | ed_lo] bf16 by pos
    h1 = x @ W1
    es1 = h1 @ as1
    ed1 = h1 @ ad1
    tbl = np.zeros((RTOT, RC), bf)
    tbl[:, F] = bf(-200.0)  # pad rows: es_eff = -200
    tbl[posn, :F] = h1.astype(bf)
    eh, el = _hi_lo(es1, bf)
    tbl[posn, F] = eh
    tbl[posn, F + 1] = el
    dh, dl = _hi_lo(ed1, bf)
    tbl[posn, F + 2] = dh
    tbl[posn, F + 3] = dl

    if cold:
        # no threaded uploads while the first jit/compile is pending
        dev_t1 = jax.device_put(tbl, shard)
        ell = _prep_edges(gp, N, src, dst)
        dev_idx = jax.device_put(
            ell.reshape(N_CORES * P, gp["TOTC"]), shard)
    else:
        # overlap the two ~13.6MB uploads with the edge prep
        from concurrent.futures import ThreadPoolExecutor
        tpool = ThreadPoolExecutor(2)
        fut_t1 = tpool.submit(lambda: jax.device_put(tbl, shard))
        ell = _prep_edges(gp, N, src, dst)
        fut_idx = tpool.submit(
            lambda: jax.device_put(ell.reshape(N_CORES * P, gp["TOTC"]),
                                   shard))
        dev_t1 = fut_t1.result()
        dev_idx = fut_idx.result()
        tpool.shutdown(wait=False)
    kernel._prep_time = time.perf_counter() - t_prep0

    t_host0 = time.perf_counter()
    w2m = np.concatenate([W2, (W2 @ as2)[:, None], (W2 @ ad2)[:, None]], 1)
    w3m = np.concatenate([W3, (W3 @ as3)[:, None], (W3 @ ad3)[:, None]], 1)
    ins = {
        "t1": dev_t1,
        "idx": dev_idx,
        "esb": gp["esb"].reshape(N_CORES * P, T_core),
        "gidx": gidx.reshape(N_CORES * P, GCOLS),
        "gfix": gfix.reshape(N_CORES * P, 2),
        "w2": np.ascontiguousarray(w2m),
        "w3": np.ascontiguousarray(w3m),
        "b1": np.tile(b1.reshape(1, F), (P, 1)),
        "b2": np.tile(b2.reshape(1, F), (P, 1)),
        "b3": np.tile(b3.reshape(1, F), (P, 1)),
    }
    kernel._host_table_time = time.perf_counter() - t_host0
    kernel._last_ins = ins

    t0 = time.perf_counter()
    out = fn(ins)  # [8*P, 2F] fp32
    kernel._launch_times = [time.perf_counter() - t0]

    t_post0 = time.perf_counter()
    pooled = out.reshape(N_CORES, P, 2 * F)[:, :GPC, :].reshape(G, 2 * F)
    pooled = pooled.copy()
    pooled[counts == 0] = 0.0  # empty graphs: reference yields 0
    kernel._post_time = time.perf_counter() - t_post0
    return (pooled @ linW + linb).astype(np.float32)


# revision 25
# speedup vs baseline: 1.5006x; 1.5006x over previous
"""GAT GNN kernel for 8 Trainium2 NeuronCores (Bass, via PJRT/axon).

Single fused launch for all three GAT layers. Strategy:
  - Nodes are permuted by in-degree so each 128-node dst tile has
    near-uniform degree -> tight ELL slot grids, shared by all layers.
  - Host computes h1 = x @ W1 and es/ed logit terms, ships per-core bf16
    table shards (row = [h(64) | es_hi | es_lo | ed_hi | ed_lo]) plus the
    ELL index grid (uint32 positions into the AllGathered table).
  - Device, per layer: AllGather table shards DRAM->DRAM across the 8
    cores (NeuronLink, not the host tunnel), per-tile indirect-DMA
    gathers, on-chip segment softmax (DVE+ACT), slot reduction, then one
    PE matmul per tile with rhs [W_next | W_next@a_src | W_next@a_dst]
    which directly yields the next layer's [h | es | ed] table shard.
  - Graph mean/max pooling also runs on device: layer-3 node features are
    AllGathered, then each core gathers its G/8 graphs' node rows (pad
    slots duplicate the graph's first node, corrected on DVE) and
    reduces sum+max on-chip. Only [G, 2F] pooled features return to the
    host, which applies the final linear.
All tables stay on device between layers; host<->device traffic is
~28 MB up + 0.5 MB down (the axon tunnel moves ~40 MB/s, so transfers
dominate wall time; device exec is ~12-16 ms, Pool-engine SWDGE bound
at ~1 us per 128-row indirect gather x ~3330 slot-columns x 3 layers).
"""
import sys

sys.path.insert(0, "/opt/trn_rl_repo")

import numpy as np

P = 128
F = 64                 # hidden width
RC = 68                # table row: 64 feats + es_hi, es_lo, ed_hi, ed_lo
RCO = 66               # matmul output row: 64 feats + es + ed (fp32)
NEG_SLOPE = 0.2
N_CORES = 8
COLS_BUDGET = 224      # gather cols per group
MAX_NT = 8
G = 512                # graphs

_RUNNERS = {}


def _bf16():
    import ml_dtypes
    return ml_dtypes.bfloat16


def _make_runner(nc, replicated_names):
    """jit the bass module over 8 cores via shard_map. Inputs in
    replicated_names get PartitionSpec(None); others shard on axis 0."""
    import jax
    from jax.sharding import Mesh, PartitionSpec
    from jax.experimental.shard_map import shard_map
    import concourse.mybir as mybir
    from concourse.bass2jax import (_bass_exec_p, partition_id_tensor,
                                    install_neuronx_cc_hook)

    install_neuronx_cc_hook()
    nc.finalize()
    partition_name = nc.partition_id_tensor.name if nc.partition_id_tensor else None

    in_names, out_names, out_avals, zero_outs = [], [], [], []
    for alloc in nc.m.functions[0].allocations:
        if not isinstance(alloc, mybir.MemoryLocationSet):
            continue
        name = alloc.memorylocations[0].name
        if alloc.kind == "ExternalInput":
            if name != partition_name:
                in_names.append(name)
        elif alloc.kind == "ExternalOutput":
            shape = tuple(alloc.tensor_shape)
            dtype = mybir.dt.np(alloc.dtype)
            out_names.append(name)
            out_avals.append(jax.core.ShapedArray(shape, dtype))
            zero_outs.append(np.zeros(shape, dtype))
    all_in = in_names + out_names + ([partition_name] if partition_name else [])

    def _body(*args):
        operands = list(args)
        if partition_name is not None:
            operands.append(partition_id_tensor())
        return tuple(_bass_exec_p.bind(
            *operands,
            out_avals=tuple(out_avals), in_names=tuple(all_in),
            out_names=tuple(out_names), lowering_input_output_aliases=(),
            sim_require_finite=False, sim_require_nnan=False, nc=nc))

    devices = jax.devices()[:N_CORES]
    mesh = Mesh(np.asarray(devices), ("core",))
    in_specs = tuple(
        PartitionSpec(None) if n in replicated_names else PartitionSpec("core")
        for n in in_names) + (PartitionSpec("core"),) * len(out_names)
    out_specs = (PartitionSpec("core"),) * len(out_names)
    jfn = jax.jit(shard_map(_body, mesh=mesh, in_specs=in_specs,
                            out_specs=out_specs, check_rep=False),
                  keep_unused=True)

    dev_zeros = []

    def fn(global_ins):
        import jax as _jax
        from jax.sharding import NamedSharding
        args = [global_ins[n] for n in in_names]
        if not dev_zeros:
            # outputs are fully written by the kernel; keep one
            # device-resident zero buffer instead of re-uploading 12.8MB
            # of zeros through the tunnel every call
            sh = NamedSharding(mesh, PartitionSpec("core"))
            for z in zero_outs:
                dev_zeros.append(_jax.device_put(
                    np.zeros((N_CORES * z.shape[0], *z.shape[1:]), z.dtype),
                    sh))
        args += dev_zeros
        outs = jfn(*args)
        _jax.block_until_ready(outs)
        return np.asarray(outs[0])

    return fn, in_names, jfn, replicated_names


def _build_fused_kernel(groups, TOTC, SHR, T_core, GCOLS):
    """All three GAT layers + graph pooling in one module, one core's
    dst shard each. Output: [P, 2F] fp32 (rows 0..G/8-1 = this core's
    graphs; cols 0..F-1 mean-pool, F..2F-1 max-pool)."""
    import concourse.bacc as bacc
    import concourse.bass as bass
    import concourse.mybir as mybir
    import concourse.tile as tile
    from concourse.masks import make_identity

    DT = mybir.dt.float32
    BF = mybir.dt.bfloat16
    A = mybir.AluOpType
    RTOT = N_CORES * SHR

    nc = bacc.Bacc("TRN2", target_bir_lowering=False, debug=False,
                   num_devices=N_CORES)
    t1_d = nc.dram_tensor("t1", [SHR, RC], BF, kind="ExternalInput")
    idx_d = nc.dram_tensor("idx", [P, TOTC], mybir.dt.uint32,
                           kind="ExternalInput")
    esb_d = nc.dram_tensor("esb", [P, T_core], DT, kind="ExternalInput")
    gidx_d = nc.dram_tensor("gidx", [P, GCOLS], mybir.dt.uint32,
                            kind="ExternalInput")
    gfix_d = nc.dram_tensor("gfix", [P, 2], DT, kind="ExternalInput")
    w2_d = nc.dram_tensor("w2", [F, RCO], DT, kind="ExternalInput")
    w3_d = nc.dram_tensor("w3", [F, RCO], DT, kind="ExternalInput")
    b1_d = nc.dram_tensor("b1", [P, F], DT, kind="ExternalInput")
    b2_d = nc.dram_tensor("b2", [P, F], DT, kind="ExternalInput")
    b3_d = nc.dram_tensor("b3", [P, F], DT, kind="ExternalInput")
    out_d = nc.dram_tensor("out", [P, 2 * F], DT, kind="ExternalOutput")

    with tile.TileContext(nc) as tc:
        with (tc.tile_pool(name="const", bufs=1) as cpool,
              tc.tile_pool(name="sb", bufs=2) as pool,
              tc.tile_pool(name="ps", bufs=2, space="PSUM") as pspool,
              tc.tile_pool(name="dS", bufs=2, space="DRAM") as dramS,
              tc.tile_pool(name="dS3", bufs=1, space="DRAM") as dramS3,
              tc.tile_pool(name="dT3", bufs=1, space="DRAM") as dramT3,
              tc.tile_pool(name="dT", bufs=2, space="DRAM") as dramT):
            ident = cpool.tile([P, P], DT)
            make_identity(nc, ident[:])
            w_sb = {}
            for lname, wd in ((2, w2_d), (3, w3_d)):
                w_sb[lname] = cpool.tile([F, RCO], DT, name=f"wsb{lname}")
                nc.sync.dma_start(out=w_sb[lname][:], in_=wd[:])
            b_sb = {}
            for lname, bd in ((1, b1_d), (2, b2_d), (3, b3_d)):
                b_sb[lname] = cpool.tile([P, F], DT, name=f"bsb{lname}")
                nc.sync.dma_start(out=b_sb[lname][:], in_=bd[:])
            esb_sb = cpool.tile([P, T_core], DT)
            nc.sync.dma_start(out=esb_sb[:], in_=esb_d[:])

            # layer-1 table: bounce the input shard, AllGather to full T
            S0 = dramS.tile([SHR, RC], BF, tag="S")
            nc.gpsimd.dma_start(out=S0[:], in_=t1_d[:])
            T_cur = dramT.tile([RTOT, RC], BF, tag="T")
            nc.gpsimd.collective_compute(
                "AllGather", A.bypass,
                replica_groups=[list(range(N_CORES))],
                ins=[S0.opt()], outs=[T_cur.opt()])

            def bcast(t, dims):
                b = t[:]
                return bass.AP(b.tensor, b.offset, [[b.ap[0][0], P]] + dims)

            S3 = None
            for layer in (1, 2, 3):
                S_next = (dramS.tile([SHR, RC], BF, tag="S", name="Snext")
                          if layer < 3 else None)
                if layer == 3:
                    S3 = dramS3.tile([SHR, F], BF, tag="S3", name="S3t")
                assert T_cur[:].offset == 0
                for (col_off, j0, nt, Kg) in groups:
                    cols = nt * Kg
                    it = pool.tile([P, cols], mybir.dt.uint32, tag="it")
                    nc.sync.dma_start(out=it[:],
                                      in_=idx_d[:, col_off:col_off + cols])
                    g = pool.tile([P, cols * RC], BF, tag="g")
                    for cc in range(cols):
                        nc.gpsimd.indirect_dma_start(
                            out=g[:, cc * RC:(cc + 1) * RC], out_offset=None,
                            in_=T_cur[:],
                            in_offset=bass.IndirectOffsetOnAxis(
                                ap=it[:, cc:cc + 1], axis=0))
                    gb = g[:]
                    pstep = gb.ap[0][0]

                    def gap(off, dims):
                        return bass.AP(gb.tensor, gb.offset + off,
                                       [[pstep, P]] + dims)

                    # z = es_hi + es_lo (slot) + ed_hi + ed_lo (self slot 0)
                    z = pool.tile([P, cols], DT, tag="z")
                    nc.vector.tensor_tensor(
                        out=z[:], in0=gap(F, [[RC, cols]]),
                        in1=gap(F + 1, [[RC, cols]]), op=A.add)
                    zed = pool.tile([P, nt], DT, tag="zed")
                    nc.vector.tensor_tensor(
                        out=zed[:], in0=gap(F + 2, [[Kg * RC, nt]]),
                        in1=gap(F + 3, [[Kg * RC, nt]]), op=A.add)
                    nc.vector.tensor_tensor(
                        out=z[:], in0=z[:],
                        in1=bcast(zed, [[1, nt], [0, Kg]]), op=A.add)
                    # leaky relu (exact): z = max(z, 0.2z), clamp at -30
                    zt = pool.tile([P, cols], DT, tag="zt")
                    nc.vector.tensor_scalar_mul(zt[:], z[:], NEG_SLOPE)
                    nc.vector.tensor_tensor(out=z[:], in0=z[:], in1=zt[:],
                                            op=A.max)
                    nc.vector.tensor_scalar_max(z[:], z[:], -30.0)
                    pex = pool.tile([P, cols], BF, tag="pex")
                    nc.scalar.activation(pex[:], z[:],
                                         mybir.ActivationFunctionType.Exp)
                    den = pool.tile([P, nt], DT, tag="den")
                    nc.vector.tensor_reduce(
                        out=den[:],
                        in_=pex[:].rearrange("p (t k) -> p t k", k=Kg),
                        axis=mybir.AxisListType.X, op=A.add)
                    nc.vector.reciprocal(den[:], den[:])
                    # feats *= p (in place, bf16)
                    nc.vector.tensor_tensor(
                        out=gap(0, [[RC, cols], [1, F]]),
                        in0=gap(0, [[RC, cols], [1, F]]),
                        in1=bcast(pex, [[1, cols], [0, F]]),
                        op=A.mult)
                    # reduce slots -> agg [P, nt*F] fp32
                    agg = pool.tile([P, nt * F], DT, tag="agg")
                    nc.vector.tensor_reduce(
                        out=agg[:],
                        in_=gap(0, [[Kg * RC, nt], [1, F], [RC, Kg]]),
                        axis=mybir.AxisListType.X, op=A.add)
                    nc.vector.tensor_tensor(
                        out=agg[:], in0=agg[:],
                        in1=bcast(den, [[1, nt], [0, F]]), op=A.mult)
                    nc.vector.tensor_tensor(
                        out=agg[:], in0=agg[:],
                        in1=bcast(b_sb[layer], [[0, nt], [1, F]]), op=A.add)

                    if layer == 3:
                        outf = pool.tile([P, nt * F], BF, tag="outf")
                        nc.vector.tensor_copy(out=outf[:], in_=agg[:])
                        sb3 = S3[:]
                        dst = bass.AP(sb3.tensor, sb3.offset + j0 * P * F,
                                      [[F, P], [P * F, nt], [1, F]])
                        nc.sync.dma_start(out=dst, in_=outf[:])
                        continue

                    # relu, transpose, matmul -> [h | es | ed] fp32
                    nc.vector.tensor_scalar_max(agg[:], agg[:], 0.0)
                    aggv = agg[:].rearrange("p (t f) -> p t f", f=F)
                    psT = pspool.tile([F, nt * P], DT, tag="psT")
                    for t in range(nt):
                        nc.tensor.transpose(out=psT[:, t * P:(t + 1) * P],
                                            in_=aggv[:, t, :],
                                            identity=ident[:])
                    aggT = pool.tile([F, nt * P], DT, tag="aggT")
                    nc.vector.tensor_copy(out=aggT[:], in_=psT[:])
                    # 128-float (512B) stride per tile so no matmul output
                    # crosses a 2KB PSUM bank boundary
                    RCOP = 128
                    psO = pspool.tile([P, nt * RCOP], DT, tag="psO")
                    for t in range(nt):
                        nc.tensor.matmul(
                            out=psO[:, t * RCOP:t * RCOP + RCO],
                            lhsT=aggT[:, t * P:(t + 1) * P],
                            rhs=w_sb[layer + 1][:], start=True, stop=True)
                    pb = psO[:]
                    pstep2 = pb.ap[0][0]

                    def pap(off, dims):
                        return bass.AP(pb.tensor, pb.offset + off,
                                       [[pstep2, P]] + dims)

                    nxt = pool.tile([P, nt * RC], BF, tag="nxt")
                    nb = nxt[:]
                    nstep = nb.ap[0][0]

                    def nap(off, dims):
                        return bass.AP(nb.tensor, nb.offset + off,
                                       [[nstep, P]] + dims)

                    # feats
                    nc.vector.tensor_copy(
                        out=nap(0, [[RC, nt], [1, F]]),
                        in_=pap(0, [[RCOP, nt], [1, F]]))
                    # es_eff = es + bias (bias -200 on pad rows); hi/lo bf16
                    esum = pool.tile([P, nt], DT, tag="esum")
                    eb = esb_sb[:]
                    nc.vector.tensor_tensor(
                        out=esum[:], in0=pap(F, [[RCOP, nt]]),
                        in1=bass.AP(eb.tensor, eb.offset + j0,
                                    [[eb.ap[0][0], P], [1, nt]]),
                        op=A.add)
                    nc.vector.tensor_copy(out=nap(F, [[RC, nt]]),
                                          in_=esum[:])
                    hif = pool.tile([P, nt], DT, tag="hif")
                    nc.vector.tensor_copy(out=hif[:], in_=nap(F, [[RC, nt]]))
                    nc.vector.tensor_tensor(
                        out=nap(F + 1, [[RC, nt]]), in0=esum[:], in1=hif[:],
                        op=A.subtract)
                    # ed hi/lo
                    nc.vector.tensor_copy(out=nap(F + 2, [[RC, nt]]),
                                          in_=pap(F + 1, [[RCOP, nt]]))
                    hif2 = pool.tile([P, nt], DT, tag="hif2")
                    nc.vector.tensor_copy(out=hif2[:],
                                          in_=nap(F + 2, [[RC, nt]]))
                    nc.vector.tensor_tensor(
                        out=nap(F + 3, [[RC, nt]]),
                        in0=pap(F + 1, [[RCOP, nt]]), in1=hif2[:],
                        op=A.subtract)
                    sb = S_next[:]
                    dst = bass.AP(sb.tensor, sb.offset + j0 * P * RC,
                                  [[RC, P], [P * RC, nt], [1, RC]])
                    nc.sync.dma_start(out=dst, in_=nxt[:])

                if layer < 3:
                    T_cur = dramT.tile([RTOT, RC], BF, tag="T")
                    nc.gpsimd.collective_compute(
                        "AllGather", A.bypass,
                        replica_groups=[list(range(N_CORES))],
                        ins=[S_next.opt()], outs=[T_cur.opt()])

            # ---- graph pooling: gather this core's graphs' node rows ----
            T3 = dramT3.tile([RTOT, F], BF, tag="T3")
            nc.gpsimd.collective_compute(
                "AllGather", A.bypass,
                replica_groups=[list(range(N_CORES))],
                ins=[S3.opt()], outs=[T3.opt()])
            assert T3[:].offset == 0
            git = pool.tile([P, GCOLS], mybir.dt.uint32, tag="git")
            nc.sync.dma_start(out=git[:], in_=gidx_d[:])
            gfix = pool.tile([P, 2], DT, tag="gfix")
            nc.sync.dma_start(out=gfix[:], in_=gfix_d[:])
            gg = pool.tile([P, GCOLS * F], BF, tag="gg")
            for cc in range(GCOLS):
                nc.gpsimd.indirect_dma_start(
                    out=gg[:, cc * F:(cc + 1) * F], out_offset=None,
                    in_=T3[:],
                    in_offset=bass.IndirectOffsetOnAxis(
                        ap=git[:, cc:cc + 1], axis=0))
            ggb = gg[:]
            gstep = ggb.ap[0][0]
            gview = bass.AP(ggb.tensor, ggb.offset,
                            [[gstep, P], [1, F], [F, GCOLS]])
            pooled = pool.tile([P, 2 * F], DT, tag="pooled")
            nc.vector.tensor_reduce(out=pooled[:, 0:F], in_=gview,
                                    axis=mybir.AxisListType.X, op=A.add)
            nc.vector.tensor_reduce(out=pooled[:, F:2 * F], in_=gview,
                                    axis=mybir.AxisListType.X, op=A.max)
            # pad slots duplicated the graph's first node (slot 0):
            # sum -= padcnt * h_first; mean = sum * recip_count
            s0f = pool.tile([P, F], DT, tag="s0f")
            nc.vector.tensor_copy(out=s0f[:], in_=gg[:, 0:F])
            gfb = gfix[:]
            nc.vector.tensor_tensor(
                out=s0f[:], in0=s0f[:],
                in1=bass.AP(gfb.tensor, gfb.offset,
                            [[gfb.ap[0][0], P], [0, F]]), op=A.mult)
            nc.vector.tensor_tensor(out=pooled[:, 0:F], in0=pooled[:, 0:F],
                                    in1=s0f[:], op=A.subtract)
            nc.vector.tensor_tensor(
                out=pooled[:, 0:F], in0=pooled[:, 0:F],
                in1=bass.AP(gfb.tensor, gfb.offset + 1,
                            [[gfb.ap[0][0], P], [0, F]]), op=A.mult)
            nc.sync.dma_start(out=out_d[:], in_=pooled[:])
    return nc


def _prep_light(N, dst):
    """Degree permutation, table positions, ELL group geometry, es bias."""
    deg = np.bincount(dst, minlength=N).astype(np.int32) + 1  # + self loop
    order = np.argsort(deg, kind="stable")
    rank = np.empty(N, np.int32)
    rank[order] = np.arange(N, dtype=np.int32)
    n_tiles = (N + P - 1) // P
    T_core = (n_tiles + N_CORES - 1) // N_CORES
    SHR = T_core * P
    RTOT = N_CORES * SHR

    # rank r -> gathered-table position
    r_all = np.arange(RTOT, dtype=np.int32)
    t_all = r_all >> 7
    posr = ((t_all % N_CORES) * SHR + (t_all // N_CORES) * P
            + (r_all & 127)).astype(np.int32)
    DUMMY = np.uint32(posr[RTOT - 1])

    degp = np.zeros(RTOT, np.int32)
    degp[:N] = deg[order]
    tile_max = degp.reshape(-1, P).max(1)
    K_hat = np.maximum(2, tile_max.reshape(T_core, N_CORES).max(1))

    groups = []
    j = 0
    col = 0
    while j < T_core:
        nt = 1
        kg = int(K_hat[j])
        while (j + nt < T_core and nt < MAX_NT
               and (nt + 1) * max(kg, int(K_hat[j + nt])) <= COLS_BUDGET):
            kg = max(kg, int(K_hat[j + nt]))
            nt += 1
        groups.append((col, j, nt, kg))
        col += nt * kg
        j += nt
    TOTC = col

    col_base = np.empty(T_core, np.int64)
    for (col_off, j0, nt, kg) in groups:
        for t in range(nt):
            col_base[j0 + t] = col_off + t * kg

    # es bias: -200 on pad rows (rank >= N)
    esb = np.zeros((N_CORES, P, T_core), np.float32)
    rp = np.arange(N, RTOT, dtype=np.int64)
    tp = rp >> 7
    esb[tp % N_CORES, rp & 127, tp // N_CORES] = -200.0

    return dict(deg=deg, order=order, rank=rank, posr=posr, T_core=T_core,
                SHR=SHR, RTOT=RTOT, groups=groups, TOTC=TOTC,
                col_base=col_base, DUMMY=DUMMY, esb=esb)


def _prep_edges(gp, N, src, dst):
    """ELL slot grid fill [cores, P, TOTC] (order within a row is free,
    so an unstable int32 argsort suffices)."""
    rank, posr, col_base = gp["rank"], gp["posr"], gp["col_base"]
    TOTC, DUMMY = gp["TOTC"], gp["DUMMY"]
    posn = posr[rank]                       # node id -> table position
    dstr = rank[dst]                        # int32
    srcp = posn[src].astype(np.uint32)
    ord_e = np.argsort(dstr)                # unstable, fast
    dstr_s = dstr[ord_e]
    srcp_s = srcp[ord_e]
    cnt = np.bincount(dstr_s, minlength=N)
    indptr = np.zeros(N + 1, np.int64)
    np.cumsum(cnt, out=indptr[1:])
    k_e = np.arange(len(dstr_s), dtype=np.int64) - indptr[dstr_s]

    ell = np.full((N_CORES, P, TOTC), DUMMY, np.uint32)
    r = np.arange(N, dtype=np.int64)
    t = r >> 7
    ell[t % N_CORES, r & 127, col_base[t // N_CORES]] = \
        posr[:N].astype(np.uint32)  # self slot of rank r
    te = dstr_s >> 7
    ell[te % N_CORES, dstr_s & 127,
        col_base[te // N_CORES] + 1 + k_e] = srcp_s
    return ell


def _prep_graph(N, src, dst):
    """Compatibility wrapper: light + edge prep merged."""
    gp = _prep_light(N, dst)
    gp["ell"] = _prep_edges(gp, N, src, dst)
    return gp


def _hi_lo(v, bf):
    hi = v.astype(bf)
    lo = (v - hi.astype(np.float32)).astype(bf)
    return hi, lo


def kernel(x, edge_index, batch, W1, as1, ad1, b1, W2, as2, ad2, b2,
           W3, as3, ad3, b3, linW, linb):
    import time
    bf = _bf16()

    x = np.asarray(x, np.float32)
    edge_index = np.asarray(edge_index)
    batch = np.asarray(batch).astype(np.int64)
    W1, W2, W3 = (np.asarray(w, np.float32) for w in (W1, W2, W3))
    as1, ad1, as2, ad2, as3, ad3 = (np.asarray(a, np.float32)
                                    for a in (as1, ad1, as2, ad2, as3, ad3))
    b1, b2, b3 = (np.asarray(b, np.float32) for b in (b1, b2, b3))
    linW = np.asarray(linW, np.float32)
    linb = np.asarray(linb, np.float32)

    N = x.shape[0]
    src = edge_index[0].astype(np.int64)
    dst = edge_index[1].astype(np.int64)

    t_prep0 = time.perf_counter()
    gp = _prep_light(N, dst)
    SHR, T_core, RTOT = gp["SHR"], gp["T_core"], gp["RTOT"]
    posr, rank = gp["posr"], gp["rank"]
    posn = posr[rank]       # node id -> table position

    # graph pooling layout: core c owns graphs [c*G/8, (c+1)*G/8)
    GPC = G // N_CORES
    starts = np.searchsorted(batch, np.arange(G))
    ends = np.searchsorted(batch, np.arange(G), side="right")
    counts = (ends - starts).astype(np.int64)
    GCOLS = max(1, int(counts.max()))
    firstpos = np.where(counts > 0,
                        posn[np.minimum(starts, N - 1)],
                        gp["DUMMY"]).astype(np.uint32)
    garr = np.tile(firstpos[:, None], (1, GCOLS))
    n_all = np.arange(N, dtype=np.int64)
    gof = batch
    garr[gof, n_all - starts[gof]] = posn.astype(np.uint32)
    gidx = np.full((N_CORES, P, GCOLS), gp["DUMMY"], np.uint32)
    gidx[:, :GPC, :] = garr.reshape(N_CORES, GPC, GCOLS)
    gfix = np.zeros((N_CORES, P, 2), np.float32)
    gfix[:, :, 0] = GCOLS  # unused rows: all-pad
    gfix[:, :GPC, 0] = (GCOLS - counts).reshape(N_CORES, GPC)
    gfix[:, :, 1] = 1.0
    gfix[:, :GPC, 1] = (1.0 / np.maximum(counts, 1)).reshape(N_CORES, GPC)

    key = (N, int(edge_index.shape[1]), gp["TOTC"], GCOLS)
    cold = key not in _RUNNERS
    if cold:
        nc = _build_fused_kernel(gp["groups"], gp["TOTC"], SHR, T_core,
                                 GCOLS)
        fn, in_names, jfn, repl = _make_runner(
            nc, {"w2", "w3", "b1", "b2", "b3"})
        _RUNNERS[key] = (fn, in_names, jfn, repl)
    fn, in_names, _, _ = _RUNNERS[key]

    import jax
    from jax.sharding import Mesh, PartitionSpec, NamedSharding
    mesh = Mesh(np.asarray(jax.devices()[:N_CORES]), ("core",))
    shard = NamedSharding(mesh, PartitionSpec("core"))

    # layer-1 table shard: [h1 | es_hi | es_lo | ed_hi | ed_lo] bf16 by pos
    h1 = x @ W1
    es1 = h1 @ as1
    ed1 = h1 @ ad1
    tbl = np.zeros((RTOT, RC), bf)
    tbl[:, F] = bf(-200.0)  # pad rows: es_eff = -200
    tbl[posn, :F] = h1.astype(bf)
    eh, el = _hi_lo(es1, bf)
    tbl[posn, F] = eh
    tbl[posn, F + 1] = el
    dh, dl = _hi_lo(ed1, bf)
    tbl[posn, F + 2] = dh
    tbl[posn, F + 3] = dl

    if cold:
        # no threaded uploads while the first jit/compile is pending
        dev_t1 = jax.device_put(tbl, shard)
        ell = _prep_edges(gp, N, src, dst)
        dev_idx = jax.device_put(
            ell.reshape(N_CORES * P, gp["TOTC"]), shard)
    else:
        # overlap the two ~13.6MB uploads with the edge prep
        from concurrent.futures import ThreadPoolExecutor
        tpool = ThreadPoolExecutor(2)
        fut_t1 = tpool.submit(lambda: jax.device_put(tbl, shard))
        ell = _prep_edges(gp, N, src, dst)
        fut_idx = tpool.submit(
            lambda: jax.device_put(ell.reshape(N_CORES * P, gp["TOTC"]),
                                   shard))
        dev_t1 = fut_t1.result()
        dev_idx = fut_idx.result()
        tpool.shutdown(wait=False)
    kernel._prep_time = time.perf_counter() - t_prep0

    t_host0 = time.perf_counter()
    w2m = np.concatenate([W2, (W2 @ as2)[:, None], (W2 @ ad2)[:, None]], 1)
    w3m = np.concatenate([W3, (W3 @ as3)[:, None], (W3 @ ad3)[:, None]], 1)
    ins = {
        "t1": dev_t1,
        "idx": dev_idx,
        "esb": gp["esb"].reshape(N_CORES * P, T_core),
        "gidx": gidx.reshape(N_CORES * P, GCOLS),
        "gfix": gfix.reshape(N_CORES * P, 2),
        "w2": np.ascontiguousarray(w2m),
        "w3": np.ascontiguousarray(w3m),
        "b1": np.tile(b1.reshape(1, F), (P, 1)),
        "b2": np.tile(b2.reshape(1, F), (P, 1)),
        "b3": np.tile(b3.reshape(1, F), (P, 1)),
    }
    kernel._host_table_time = time.perf_counter() - t_host0
    kernel._last_ins = ins

    t0 = time.perf_counter()
    out = fn(ins)  # [8*P, 2F] fp32
    kernel._launch_times = [time.perf_counter() - t0]

    t_post0 = time.perf_counter()
    pooled = out.reshape(N_CORES, P, 2 * F)[:, :GPC, :].reshape(G, 2 * F)
    pooled = pooled.copy()
    pooled[counts == 0] = 0.0  # empty graphs: reference yields 0
    kernel._post_time = time.perf_counter() - t_post0
    return (pooled @ linW + linb).astype(np.float32)
